# revision 1
# baseline (speedup 1.0000x reference)
"""CenterlineDiceLoss (soft-skeleton clDice) Trainium2 Bass kernel.

Strategy: data-parallel over the batch (8 images -> 8 NeuronCores).  Each
core computes the two soft skeletons (sigmoid(pred), target) of its image
entirely SBUF-resident in fp16, using the identity that the erosion inside
``open(e_i)`` *is* ``e_{i+1}``, so each of the 11 rounds needs one 3x3 min
pool + one 3x3 max pool (separable, pairwise decomposition).  The skel
recurrence is tracked in complement space w = 1 - skel, which turns the
relu-laden update into  w *= (1 + (o - e))  (two fused scalar_tensor_tensor
ops), and the final four global sums reduce on-chip to 6 scalars per core
that the host combines into the loss.

Layout: image row 8p+j lives on partition p at free slot (j, c); all DVE
operands are kept 4B-aligned (shifted reads go through ScalarE copies) so
fp16 tensor_tensor runs in the 2x perf mode.  Vertical pooling crosses
partitions only at the 2 boundary rows per partition, exchanged with small
SBUF->SBUF DMAs.
"""

import os
import numpy as np

NUM_ITER = 10
SMOOTH = 1.0
EPS = 1e-7
SENT = 30000.0  # pad sentinel (exactly representable in fp16)

_BUILT = {}


def _install_walrus_wait_patch():
    """This container's walrus rejects >1 sync-wait per instruction; split
    extra waits onto NoOp/Drain instructions on the same engine."""
    import concourse.tile as tile_mod
    import mybir

    if getattr(tile_mod.TileContext, "_cldice_patched", False):
        return

    _orig_add_instruction = tile_mod.TileContext._add_instruction
    _ctr = [0]

    def _patched_add_instruction(self, inst):
        si = getattr(inst, "sync_info", None)
        if (
            si is not None
            and si.on_wait is not None
            and len(si.on_wait) > 1
            and inst.engine != mybir.EngineType.Unassigned
        ):
            waits = list(si.on_wait)
            ups = list(si.on_update) if si.on_update else []
            for w in waits[:-1]:
                _ctr[0] += 1
                nop = mybir.InstNoOp(
                    name=f"{inst.name}_sw{_ctr[0]}",
                    sync_info=mybir.SyncInfo(on_wait=[w], on_update=[]),
                    bass_nofuse=True,
                    engine=inst.engine,
                )
                _orig_add_instruction(self, nop)
            inst.sync_info = mybir.SyncInfo(on_wait=waits[-1:], on_update=ups)
        return _orig_add_instruction(self, inst)

    def _patched_drain_and_barrier(self, tick_clock, wait_clock):
        nc = self.nc
        drain_inst = nc.sync.drain()
        wait_clock.add_sem_waits(
            drain_inst.ins, tile_mod.ScopedClock({None: tick_clock.global_clock})
        )
        si = drain_inst.ins.sync_info
        if si is not None and si.on_wait is not None and len(si.on_wait) > 1:
            waits = list(si.on_wait)
            ups = list(si.on_update) if si.on_update else []
            drain_inst.ins.sync_info = mybir.SyncInfo(on_wait=waits[:1], on_update=[])
            for w in waits[1:]:
                extra = nc.sync.drain()
                extra.ins.sync_info = mybir.SyncInfo(on_wait=[w], on_update=[])
            if ups:
                extra2 = nc.sync.drain()
                extra2.ins.sync_info = mybir.SyncInfo(on_wait=[], on_update=ups)
        nc.all_engine_barrier()
        assert self.sems is not None
        popped = nc._tile_sem_poison_stack.pop()
        assert popped is self._sem_poison
        nc.clear_and_free_semaphores(list(self.sems.allocated().values()))
        nc.all_engine_barrier()

    tile_mod.TileContext._add_instruction = _patched_add_instruction
    tile_mod.TileContext._drain_and_barrier = _patched_drain_and_barrier
    tile_mod.TileContext._cldice_patched = True


def build_nc(H=1024, W=1024, dtname="fp16", rounds=NUM_ITER + 1):
    """Build the single-core Bass program (run SPMD across 8 cores)."""
    import concourse.bass as bass
    import concourse.bass_isa as bass_isa
    import concourse.tile as tile
    import mybir

    _install_walrus_wait_patch()

    P = 128
    R = H // P          # image rows per partition
    WB = W + 4          # padded row width (2 sentinel cols each side)
    T = W // 2          # column strip width
    NS = W // T
    fp32 = mybir.dt.float32
    dt = {"fp16": mybir.dt.float16, "fp32": mybir.dt.float32}[dtname]
    AL = mybir.AluOpType
    AF = mybir.ActivationFunctionType
    use_shift_copies = bool(os.environ.get("CLDICE_SHIFTCOPY"))

    nc = bass.Bass("TRN2", target_bir_lowering=False, debug=False)
    pred_d = nc.dram_tensor("pred", [H, W], dt, kind="ExternalInput").ap()
    targ_d = nc.dram_tensor("target", [H, W], dt, kind="ExternalInput").ap()
    p16_d = nc.dram_tensor("p16", [P, R, W], dt).ap()
    t16_d = nc.dram_tensor("t16", [P, R, W], dt).ap()
    out_d = nc.dram_tensor("out", [1, 16], fp32, kind="ExternalOutput").ap()

    with tile.TileContext(nc) as tc:
        with tc.tile_pool(name="persist", bufs=1) as pp:
            # persistent state, per chain (0 = pred-prob skeleton, 1 = target)
            eA, eB, wbuf = [], [], []
            for ch in range(2):
                eA.append(pp.tile([P, R, WB], dt, tag=f"eA{ch}", name=f"eA{ch}"))
                eB.append(pp.tile([P, R, WB], dt, tag=f"eB{ch}", name=f"eB{ch}"))
                wbuf.append(pp.tile([P, R, W], dt, tag=f"w{ch}", name=f"w{ch}"))
            consts = pp.tile([P, 2, T], dt, tag="consts")  # 0: +SENT, 1: -SENT
            accs = pp.tile([P, 16], fp32, tag="accs")
            redout = pp.tile([P, 16], fp32, tag="redout")

            ones = pp.tile([P, 1], fp32, tag="ones", name="ones")
            nc.vector.memset(ones[:], 1.0)
            nc.vector.memset(accs[:], 0.0)
            nc.vector.memset(consts[:, 0:1, :], SENT)
            nc.vector.memset(consts[:, 1:2, :], -SENT)

            # ---------------- init: load f32, sigmoid/cast to fp16 ----------
            # strip-wise to keep the f32 staging small; accumulate E/F sums.
            with tc.tile_pool(name="init", bufs=2) as ip:
                for ch, (src_d, func, acc_col) in enumerate(
                    [(pred_d, AF.Sigmoid, 5), (targ_d, AF.Copy, 4)]
                ):
                    src_r = src_d.rearrange("(p j) c -> p j c", p=P)
                    for s in range(NS):
                        cs = T * s
                        tmp32 = ip.tile([P, R, T], dt, tag="tmp32")
                        nc.sync.dma_start(tmp32[:], src_r[:, :, cs : cs + T])
                        col = acc_col if s == NS - 1 else 8 + s
                        nc.scalar.activation(
                            eA[ch][:, :, cs + 2 : cs + T + 2],
                            tmp32[:],
                            func,
                            accum_out=accs[:, col : col + 1],
                        )
                    # combine strip partials into the final E/F column
                    for s in range(NS - 1):
                        nc.vector.tensor_tensor(
                            out=accs[:, acc_col : acc_col + 1],
                            in0=accs[:, acc_col : acc_col + 1],
                            in1=accs[:, 8 + s : 9 + s],
                            op=AL.add,
                        )
                    dst_d = p16_d if ch == 0 else t16_d
                    nc.sync.dma_start(dst_d[:], eA[ch][:, :, 2 : W + 2])
                    # sentinel pads: eA starts as min-source (+S); eB's first
                    # role is max-source (-S)
                    nc.vector.memset(eA[ch][:, :, 0:2], SENT)
                    nc.vector.memset(eA[ch][:, :, W + 2 : W + 4], SENT)
                    nc.vector.memset(eB[ch][:, :, 0:2], -SENT)
                    nc.vector.memset(eB[ch][:, :, W + 2 : W + 4], -SENT)

            with tc.tile_pool(name="scr", bufs=2) as scr:

                # ---------------- pool pass helper --------------------------
                def pool_pass(op, src, dst_of_strip, cidx):
                    """3x3 min/max pool of `src` (padded [P,R,WB]).
                    dst_of_strip(s) -> output AP [P,R,T] for strip s.
                    cidx: 0 for min (+S halo edge), 1 for max (-S)."""
                    for s in range(NS):
                        cs = T * s
                        if use_shift_copies:
                            xs = scr.tile([P, R, T + 2], dt, tag="xs")
                            nc.scalar.activation(
                                xs[:], src[:, :, cs + 1 : cs + T + 3], AF.Copy
                            )
                            in1_h1 = xs[:]
                        else:
                            in1_h1 = src[:, :, cs + 1 : cs + T + 3]
                        m1 = scr.tile([P, R, T + 2], dt, tag="m1")
                        nc.vector.tensor_tensor(
                            out=m1[:],
                            in0=src[:, :, cs : cs + T + 2],
                            in1=in1_h1,
                            op=op,
                        )
                        if use_shift_copies:
                            m1s = scr.tile([P, R, T], dt, tag="m1s")
                            nc.scalar.activation(m1s[:], m1[:, :, 1 : T + 1], AF.Copy)
                            in1_h2 = m1s[:]
                        else:
                            in1_h2 = m1[:, :, 1 : T + 1]
                        h = scr.tile([P, R + 2, T], dt, tag="h")
                        nc.vector.tensor_tensor(
                            out=h[:, 1 : R + 1, :],
                            in0=m1[:, :, 2 : T + 2],
                            in1=in1_h2,
                            op=op,
                        )
                        # row halo exchange across partitions
                        if os.environ.get("CLDICE_NO_HALO"):
                            nc.vector.memset(h[:, 0:1, :], 0.0)
                            nc.vector.memset(h[:, R + 1 : R + 2, :], 0.0)
                            m2 = scr.tile([P, R + 1, T], dt, tag="m2")
                            nc.vector.tensor_tensor(
                                out=m2[:],
                                in0=h[:, 0 : R + 1, :],
                                in1=h[:, 1 : R + 2, :],
                                op=op,
                            )
                            nc.vector.tensor_tensor(
                                out=dst_of_strip(s),
                                in0=m2[:, 0:R, :],
                                in1=m2[:, 1 : R + 1, :],
                                op=op,
                            )
                            continue
                        nc.sync.dma_start(h[1:P, 0:1, :], h[0 : P - 1, R : R + 1, :])
                        nc.sync.dma_start(
                            h[0:1, 0:1, :], consts[0:1, cidx : cidx + 1, :]
                        )
                        nc.sync.dma_start(
                            h[0 : P - 1, R + 1 : R + 2, :], h[1:P, 1:2, :]
                        )
                        nc.sync.dma_start(
                            h[P - 1 : P, R + 1 : R + 2, :],
                            consts[0:1, cidx : cidx + 1, :],
                        )
                        m2 = scr.tile([P, R + 1, T], dt, tag="m2")
                        nc.vector.tensor_tensor(
                            out=m2[:],
                            in0=h[:, 0 : R + 1, :],
                            in1=h[:, 1 : R + 2, :],
                            op=op,
                        )
                        nc.vector.tensor_tensor(
                            out=dst_of_strip(s),
                            in0=m2[:, 0:R, :],
                            in1=m2[:, 1 : R + 1, :],
                            op=op,
                        )

                # ---------------- skeleton rounds ---------------------------
                cur = [eA[0], eA[1]]
                nxt = [eB[0], eB[1]]
                for i in range(rounds):
                    for ch in range(2):
                        pool_pass(
                            AL.min,
                            cur[ch],
                            lambda s, ch=ch: nxt[ch][
                                :, :, T * s + 2 : T * s + T + 2
                            ],
                            0,
                        )
                    for ch in range(2):
                        o_strips = [None] * NS

                        def max_dst(s, o_strips=o_strips):
                            o = scr.tile([P, R, T], dt, tag="o", name="o")
                            o_strips[s] = o
                            return o[:]

                        pool_pass(AL.max, nxt[ch], max_dst, 1)
                        # nxt's pads served the max pass (-S); flip to +S for
                        # its next life as min-source
                        nc.gpsimd.memset(nxt[ch][:, :, 0:2], SENT)
                        nc.gpsimd.memset(nxt[ch][:, :, W + 2 : W + 4], SENT)
                        for s in range(NS):
                            cs = T * s
                            t_s = scr.tile([P, R, T], dt, tag="xs")
                            nc.vector.scalar_tensor_tensor(
                                out=t_s[:],
                                in0=cur[ch][:, :, cs + 2 : cs + T + 2],
                                scalar=-1.0,
                                op0=AL.mult,
                                in1=o_strips[s][:],
                                op1=AL.add,
                            )
                            if i == 0:
                                nc.scalar.activation(
                                    wbuf[ch][:, :, cs : cs + T],
                                    t_s[:],
                                    AF.Copy,
                                    bias=1.0,
                                )
                            else:
                                nc.vector.scalar_tensor_tensor(
                                    out=wbuf[ch][:, :, cs : cs + T],
                                    in0=t_s[:],
                                    scalar=1.0,
                                    op0=AL.add,
                                    in1=wbuf[ch][:, :, cs : cs + T],
                                    op1=AL.mult,
                                )
                        # cur's pads (+S, was min-source) -> -S for its next
                        # life as max-source (it becomes nxt after the swap)
                        if i < rounds - 1:
                            nc.gpsimd.memset(cur[ch][:, :, 0:2], -SENT)
                            nc.gpsimd.memset(cur[ch][:, :, W + 2 : W + 4], -SENT)
                    cur, nxt = nxt, cur

                # ---------------- final sums --------------------------------
                # accs cols: 0:A=sum(w_p*t16) 1:B=sum(w_p) 2:C=sum(w_t*p16)
                #            3:D=sum(w_t)     4:E=sum(t16) 5:F=sum(p16)
                for ch in range(2):
                    other_d = t16_d if ch == 0 else p16_d
                    prod_col = 0 if ch == 0 else 2
                    wsum_col = 1 if ch == 0 else 3
                    pc = [8 + 4 * ch + s for s in range(NS)]
                    wc = [10 + 4 * ch + s for s in range(NS)]
                    for s in range(NS):
                        cs = T * s
                        ob = scr.tile([P, R, T], dt, tag="h")
                        nc.sync.dma_start(ob[:], other_d[:, :, cs : cs + T])
                        junk = scr.tile([P, R, T], dt, tag="m2")
                        nc.vector.tensor_tensor(
                            out=junk[:],
                            in0=wbuf[ch][:, :, cs : cs + T],
                            in1=ob[:],
                            op=AL.mult,
                        )
                        nc.vector.tensor_reduce(
                            out=accs[:, pc[s] : pc[s] + 1],
                            in_=junk[:],
                            axis=mybir.AxisListType.XY,
                            op=AL.add,
                        )
                        junk2 = scr.tile([P, R, T], dt, tag="o")
                        nc.scalar.activation(
                            junk2[:],
                            wbuf[ch][:, :, cs : cs + T],
                            AF.Copy,
                            accum_out=accs[:, wc[s] : wc[s] + 1],
                        )
                    nc.vector.tensor_tensor(
                        out=accs[:, prod_col : prod_col + 1],
                        in0=accs[:, pc[0] : pc[0] + 1],
                        in1=accs[:, pc[1] : pc[1] + 1],
                        op=AL.add,
                    )
                    nc.vector.tensor_tensor(
                        out=accs[:, wsum_col : wsum_col + 1],
                        in0=accs[:, wc[0] : wc[0] + 1],
                        in1=accs[:, wc[1] : wc[1] + 1],
                        op=AL.add,
                    )

                with tc.tile_pool(name="psum", bufs=1, space="PSUM") as psp:
                    ps = psp.tile([1, 16], fp32, name="ps")
                    nc.tensor.matmul(ps[:], ones[:], accs[:], start=True, stop=True)
                    nc.vector.tensor_copy(redout[0:1, :], ps[:])
                nc.sync.dma_start(out_d[:], redout[0:1, :])

    return nc


def _get_built(H=1024, W=1024, dtname=None):
    if dtname is None:
        dtname = os.environ.get("CLDICE_DT", "fp16")
    key = (H, W, dtname)
    if key not in _BUILT:
        _BUILT[key] = build_nc(H, W, dtname)
    return _BUILT[key]


_last_run_wall = [None]


def kernel(pred: np.ndarray, target: np.ndarray) -> np.ndarray:
    """Full-input entry point: pred/target [8,1,1024,1024] f32 -> scalar."""
    import time
    from concourse.bass_utils import run_bass_kernel_spmd

    n_cores = pred.shape[0]
    dtname = os.environ.get("CLDICE_DT", "fp16")
    _np_in_dt = np.float16 if dtname == "fp16" else np.float32
    nc = _get_built(pred.shape[2], pred.shape[3], dtname)
    in_maps = [
        {
            "pred": np.ascontiguousarray(pred[c, 0], dtype=_np_in_dt),
            "target": np.ascontiguousarray(target[c, 0], dtype=_np_in_dt),
        }
        for c in range(n_cores)
    ]
    t0 = time.time()
    res = run_bass_kernel_spmd(nc, in_maps, list(range(n_cores)))
    _last_run_wall[0] = time.time() - t0
    outs = np.stack([res.results[c]["out"][0] for c in range(n_cores)])  # [8,16]
    return _combine(outs, pred.shape[2] * pred.shape[3])


def _combine(outs: np.ndarray, n_per_core: int) -> np.ndarray:
    o = outs.astype(np.float64)
    A, B, C, D, E, F = (o[:, k] for k in range(6))
    S1 = np.sum(E - A)  # sum(skel_pred * target)
    S2 = np.sum(n_per_core - B)  # sum(skel_pred)
    S3 = np.sum(F - C)  # sum(skel_target * pred_prob)
    S4 = np.sum(n_per_core - D)  # sum(skel_target)
    tprec = (S1 + SMOOTH) / (S2 + SMOOTH)
    tsens = (S3 + SMOOTH) / (S4 + SMOOTH)
    cl_dice = 2.0 * tprec * tsens / (tprec + tsens + EPS)
    return np.float32(1.0 - cl_dice)



# revision 4
# speedup vs baseline: 60.9131x; 60.9131x over previous
"""CenterlineDiceLoss (soft-skeleton clDice) Trainium2 Bass kernel, v2.

Data-parallel over the batch (8 images -> 8 NeuronCores).  Each core runs
both soft-skeleton chains (sigmoid(pred), target) fully SBUF-resident in
fp16 with ZERO DMAs inside the iteration loop:

 - The two chains are fused along the free dimension ([P, CH=2, R, W]
   tiles) so every engine instruction processes both chains at once.
 - 3x3 min/max pools are separable pairwise ops on DVE.  The one-element
   shifted operands read misaligned fp16 directly: measured on HW this
   costs only ~40% over the aligned 2x mode and beats staging shifted
   copies through ScalarE/DMA (those serialize the dependency chain).
   Edge/interior op splitting keeps the PE/ACT halo path off the DVE
   critical path.
 - Cross-partition row halos are produced by PE shift-matmuls
   (permutation matrices built once with affine_select) into PSUM and
   evacuated by ACT - no SBUF->SBUF partition-shifted DMAs at all.
 - The skeleton recurrence is tracked in complement space w = 1 - skel:
   w *= (1 + o - e), computed as st = o - e (DVE), mt = st + 1 (ACT bias
   copy), w *= mt (DVE).  (The Pool engine's tensor_tensor measured too
   slow on HW to be worth offloading to, and its scalar_tensor_tensor is
   rejected by the hardware ISA.)
 - Final global sums reduce on-chip (ACT accum + DVE reduce + PE
   ones-matmul) to one [1, 32] fp32 vector per core; the host combines
   the 8 vectors into the scalar loss.  The input images (needed for the
   cross products) are re-streamed from DRAM at that point rather than
   held in SBUF through the rounds.
"""

import os
import numpy as np

NUM_ITER = 10
SMOOTH = 1.0
EPS = 1e-7
SENT = 30000.0  # pad sentinel (exactly representable in fp16)

_BUILT = {}


def _install_walrus_wait_patch():
    """This container's walrus rejects >1 sync-wait per instruction; split
    extra waits onto NoOp/Drain instructions on the same engine."""
    import concourse.tile as tile_mod
    import mybir

    if getattr(tile_mod.TileContext, "_cldice_patched", False):
        return

    _orig_add_instruction = tile_mod.TileContext._add_instruction
    _ctr = [0]

    def _patched_add_instruction(self, inst):
        si = getattr(inst, "sync_info", None)
        if (
            si is not None
            and si.on_wait is not None
            and len(si.on_wait) > 1
            and inst.engine != mybir.EngineType.Unassigned
        ):
            waits = list(si.on_wait)
            ups = list(si.on_update) if si.on_update else []
            for w in waits[:-1]:
                _ctr[0] += 1
                nop = mybir.InstNoOp(
                    name=f"{inst.name}_sw{_ctr[0]}",
                    sync_info=mybir.SyncInfo(on_wait=[w], on_update=[]),
                    bass_nofuse=True,
                    engine=inst.engine,
                )
                _orig_add_instruction(self, nop)
            inst.sync_info = mybir.SyncInfo(on_wait=waits[-1:], on_update=ups)
        return _orig_add_instruction(self, inst)

    def _patched_drain_and_barrier(self, tick_clock, wait_clock):
        nc = self.nc
        drain_inst = nc.sync.drain()
        wait_clock.add_sem_waits(
            drain_inst.ins, tile_mod.ScopedClock({None: tick_clock.global_clock})
        )
        si = drain_inst.ins.sync_info
        if si is not None and si.on_wait is not None and len(si.on_wait) > 1:
            waits = list(si.on_wait)
            ups = list(si.on_update) if si.on_update else []
            drain_inst.ins.sync_info = mybir.SyncInfo(on_wait=waits[:1], on_update=[])
            for w in waits[1:]:
                extra = nc.sync.drain()
                extra.ins.sync_info = mybir.SyncInfo(on_wait=[w], on_update=[])
            if ups:
                extra2 = nc.sync.drain()
                extra2.ins.sync_info = mybir.SyncInfo(on_wait=[], on_update=ups)
        nc.all_engine_barrier()
        assert self.sems is not None
        popped = nc._tile_sem_poison_stack.pop()
        assert popped is self._sem_poison
        nc.clear_and_free_semaphores(list(self.sems.allocated().values()))
        nc.all_engine_barrier()

    tile_mod.TileContext._add_instruction = _patched_add_instruction
    tile_mod.TileContext._drain_and_barrier = _patched_drain_and_barrier
    tile_mod.TileContext._cldice_patched = True


def build_nc(H=1024, W=1024, rounds=NUM_ITER + 1, repeat=1, T=None):
    """Build the single-core Bass program (run SPMD across 8 cores)."""
    import concourse.bass as bass
    import concourse.tile as tile
    import mybir

    _install_walrus_wait_patch()

    P = 128
    R = H // P          # image rows per partition (8)
    CH = 2              # fused chains: 0 = sigmoid(pred), 1 = target
    WB = W + 4          # padded row: cols 0..1 pad, 2..W+1 image, W+2..W+3 pad
    if T is None:
        T = int(os.environ.get("CLDICE_T", "256"))
    NS = W // T
    fp32 = mybir.dt.float32
    dt = mybir.dt.float16
    AL = mybir.AluOpType
    AF = mybir.ActivationFunctionType

    ACCW = 8 + 6 * NS   # strip-partial columns: A,B,C,D,E,F groups of NS
    nc = bass.Bass("TRN2", target_bir_lowering=False, debug=False)
    pred_d = nc.dram_tensor("pred", [H, W], dt, kind="ExternalInput").ap()
    targ_d = nc.dram_tensor("target", [H, W], dt, kind="ExternalInput").ap()
    out_d = nc.dram_tensor("out", [1, ACCW], fp32, kind="ExternalOutput").ap()
    pred_r = pred_d.rearrange("(p j) c -> p j c", p=P)
    targ_r = targ_d.rearrange("(p j) c -> p j c", p=P)

    with tile.TileContext(nc) as tc:
        with tc.tile_pool(name="persist", bufs=1) as pp:
            eA = pp.tile([P, CH, R, WB], dt, tag="eA", name="eA")
            eB = pp.tile([P, CH, R, WB], dt, tag="eB", name="eB")
            wbuf = pp.tile([P, CH, R, W], dt, tag="w", name="w")
            accs = pp.tile([P, ACCW], fp32, tag="accs")
            redout = pp.tile([P, ACCW], fp32, tag="redout")
            ones = pp.tile([P, 1], fp32, tag="ones", name="ones")
            ones16 = pp.tile([P, P], dt, tag="ones16", name="ones16")
            # shift matrices (lhsT for matmul: out = lhsT.T @ rhs):
            # sd: out[m] = rhs[m-1]  (halo_top[p] <- row from partition p-1)
            # su: out[m] = rhs[m+1]  (halo_bot[p] <- row from partition p+1)
            sd = pp.tile([P, P], dt, tag="sd", name="sd")
            su = pp.tile([P, P], dt, tag="su", name="su")
            # edge-sentinel matmul operands: eT has a single 1 at (k=0, m=0),
            # eB at (k=0, m=127); eT.T @ sentX adds sentinel into out row 0
            # (partition 0), eB.T @ sentX into partition 127.
            eT = pp.tile([P, P], dt, tag="eT", name="eT")
            eB_m = pp.tile([P, P], dt, tag="eBm", name="eBm")
            sentP = pp.tile([P, T_MAX := 512], dt, tag="sentP", name="sentP")
            sentN = pp.tile([P, T_MAX], dt, tag="sentN", name="sentN")

            nc.vector.memset(ones[:], 1.0)
            nc.vector.memset(ones16[:], 1.0)
            nc.vector.memset(sentP[:], SENT)
            nc.vector.memset(sentN[:], -SENT)
            # lhsT[k, m] = 1 iff m == k+1   (iota = -1 - k + m == 0)
            nc.gpsimd.affine_select(
                sd[:], ones16[:], pattern=[[1, P]], compare_op=AL.is_equal,
                fill=0.0, base=-1, channel_multiplier=-1,
            )
            # lhsT[k, m] = 1 iff m == k-1   (iota = 1 - k + m == 0)
            nc.gpsimd.affine_select(
                su[:], ones16[:], pattern=[[1, P]], compare_op=AL.is_equal,
                fill=0.0, base=1, channel_multiplier=-1,
            )
            # 1 iff k + m == 0  (only k=0, m=0)
            nc.gpsimd.affine_select(
                eT[:], ones16[:], pattern=[[1, P]], compare_op=AL.is_equal,
                fill=0.0, base=0, channel_multiplier=1,
            )
            # 1 iff 127 + k - m == 0  (only k=0, m=127)
            nc.gpsimd.affine_select(
                eB_m[:], ones16[:], pattern=[[-1, P]], compare_op=AL.is_equal,
                fill=0.0, base=P - 1, channel_multiplier=1,
            )

            scr1_bufs = int(os.environ.get("CLDICE_SCR1_BUFS", "2"))
            with tc.tile_pool(name="scr1", bufs=scr1_bufs) as scr1, \
                 tc.tile_pool(name="scr2", bufs=2) as scr2, \
                 tc.tile_pool(name="mtp", bufs=int(os.environ.get("CLDICE_MTP", "2"))) as mtp, \
                 tc.tile_pool(name="psum", bufs=2 if CH * T <= 512 else 1,
                              space="PSUM") as psp:

                xs_dma = os.environ.get("CLDICE_XS", "none") == "dma"
                h_odd = os.environ.get("CLDICE_H", "m1s") == "odd"
                pipe = os.environ.get("CLDICE_PIPE", "0") == "1"

                def pool_pass(op, src, dst_of_strip, sent, post=None):
                    """3x3 pool of padded `src` [P,CH,R,WB] with `op`;
                    dst_of_strip(s) -> [P,CH,R,T] output AP for strip s.
                    `sent`: sentinel for the out-of-image row halos.
                    `post(s)`: emitted after strip s's output is ready.
                    Emission is optionally software-pipelined (stage A =
                    xs+m1, stage B = rest) so DVE has ready work while the
                    shifted copies land."""
                    stash = {}

                    def stage_a(s):
                        cs = T * s
                        m1 = scr2.tile([P, CH, R, T + 2], dt, tag="m1", name="m1")
                        # horizontal: out[c] = op(x[c-1], x[c], x[c+1])
                        xs_mode = os.environ.get("CLDICE_XS", "none")
                        if xs_mode == "none":
                            # direct misaligned read (~0.85 cyc/elem on HW)
                            xs = None
                            nc.vector.tensor_tensor(
                                out=m1[:], in0=src[:, :, :, cs : cs + T + 2],
                                in1=src[:, :, :, cs + 1 : cs + T + 3], op=op,
                            )
                        else:
                            xs = scr2.tile([P, CH, R, T + 2], dt, tag="xs",
                                           name="xs")
                            if xs_mode == "dma":
                                nc.sync.dma_start(
                                    xs[:], src[:, :, :, cs + 1 : cs + T + 3]
                                )
                            else:
                                nc.scalar.activation(
                                    xs[:], src[:, :, :, cs + 1 : cs + T + 3],
                                    AF.Copy,
                                )
                            nc.vector.tensor_tensor(
                                out=m1[:], in0=src[:, :, :, cs : cs + T + 2],
                                in1=xs[:], op=op,
                            )
                        stash[s] = (xs, m1)

                    def stage_b(s):
                        xs, m1 = stash.pop(s)
                        h = scr1.tile([P, CH, R + 2, T], dt, tag="h", name="h")
                        u = scr1.tile([P, CH, R + 1, T], dt, tag="u", name="u")
                        m1s_mode = os.environ.get("CLDICE_M1S", "none")
                        usplit = int(os.environ.get("CLDICE_USPLIT", "2"))
                        if m1s_mode == "none" and usplit >= 2:
                            # edge rows {1, R} first so the PE halo matmuls
                            # start while the interior rows compute
                            nc.vector.tensor_tensor(
                                out=h[:, :, 1 : R + 1 : R - 1, :],
                                in0=m1[:, :, 0 : R : R - 1, 2 : T + 2],
                                in1=m1[:, :, 0 : R : R - 1, 1 : T + 1], op=op,
                            )
                            nc.vector.tensor_tensor(
                                out=h[:, :, 2 : R, :],
                                in0=m1[:, :, 1 : R - 1, 2 : T + 2],
                                in1=m1[:, :, 1 : R - 1, 1 : T + 1], op=op,
                            )
                        elif m1s_mode == "none":
                            # direct misaligned read of m1
                            nc.vector.tensor_tensor(
                                out=h[:, :, 1 : R + 1, :],
                                in0=m1[:, :, :, 2 : T + 2],
                                in1=m1[:, :, :, 1 : T + 1], op=op,
                            )
                        elif h_odd:
                            nc.vector.tensor_tensor(
                                out=h[:, :, 1 : R + 1, :],
                                in0=m1[:, :, :, 1 : T + 1],
                                in1=xs[:, :, :, 2 : T + 2], op=op,
                            )
                        else:
                            m1s = scr2.tile([P, CH, R, T], dt, tag="m1s",
                                            name="m1s")
                            if os.environ.get("CLDICE_M1S", "none") == "dma":
                                nc.sync.dma_start(
                                    m1s[:, 0, :, :], m1[:, 0, :, 1 : T + 1]
                                )
                                nc.sync.dma_start(
                                    m1s[:, 1, :, :], m1[:, 1, :, 1 : T + 1]
                                )
                            else:
                                nc.scalar.activation(
                                    m1s[:], m1[:, :, :, 1 : T + 1], AF.Copy
                                )
                            nc.vector.tensor_tensor(
                                out=h[:, :, 1 : R + 1, :],
                                in0=m1[:, :, :, 2 : T + 2], in1=m1s[:], op=op,
                            )
                        return h, u

                    def stage_rest(s, h, u):
                        cs = T * s
                        # cross-partition halo rows via PE shift-matmuls,
                        # both channels per matmul (out free = CH*T <= 512).
                        # hp[:, 0] = top halo, hp[:, 1] = bottom; a second
                        # accumulating matmul adds the sentinel into the
                        # image-edge partitions (0 / 127); ACT evacuates
                        # into h rows {0, R+1}.
                        sent_t = sentP if sent > 0 else sentN
                        hp = psp.tile([P, 2, CH, T], fp32, tag="hp", name="hp")
                        if CH * T <= 512:
                            mm_groups = [(hp[:, 0, :, :], sd, h[:, :, R : R + 1, :]),
                                         (hp[:, 1, :, :], su, h[:, :, 1:2, :])]
                            sent_mats = [eT, eB_m]
                            for (dst, mat, src_rows), emat in zip(
                                mm_groups, sent_mats
                            ):
                                nc.tensor.matmul(
                                    dst, mat[:], src_rows, start=True, stop=False
                                )
                                nc.tensor.matmul(
                                    dst, emat[:], sent_t[:, 0 : CH * T],
                                    start=False, stop=True,
                                )
                        else:
                            for ch in range(CH):
                                for d, mat, emat, row in (
                                    (0, sd, eT, R), (1, su, eB_m, 1),
                                ):
                                    nc.tensor.matmul(
                                        hp[:, d, ch, :], mat[:],
                                        h[:, ch, row : row + 1, :],
                                        start=True, stop=False,
                                    )
                                    nc.tensor.matmul(
                                        hp[:, d, ch, :], emat[:],
                                        sent_t[:, 0:T],
                                        start=False, stop=True,
                                    )
                        nc.scalar.activation(h[:, :, 0:1, :], hp[:, 0, :, :], AF.Copy)
                        nc.scalar.activation(
                            h[:, :, R + 1 : R + 2, :], hp[:, 1, :, :], AF.Copy
                        )
                        # vertical: out[r] = op(h[r-1], h[r], h[r+1])
                        usplit2 = int(os.environ.get("CLDICE_USPLIT", "2"))
                        if usplit2 >= 1:
                            # interior rows don't need the halos -> no PE/ACT
                            # wait on the critical path
                            nc.vector.tensor_tensor(
                                out=u[:, :, 1:R, :], in0=h[:, :, 1:R, :],
                                in1=h[:, :, 2 : R + 1, :], op=op,
                            )
                            nc.vector.tensor_tensor(
                                out=u[:, :, 0 : R + 1 : R, :],
                                in0=h[:, :, 0 : R + 1 : R, :],
                                in1=h[:, :, 1 : R + 2 : R, :], op=op,
                            )
                        else:
                            nc.vector.tensor_tensor(
                                out=u[:], in0=h[:, :, 0 : R + 1, :],
                                in1=h[:, :, 1 : R + 2, :], op=op,
                            )
                        nc.vector.tensor_tensor(
                            out=dst_of_strip(s), in0=u[:, :, 0:R, :],
                            in1=u[:, :, 1 : R + 1, :], op=op,
                        )
                        if post is not None:
                            post(s)

                    if pipe:
                        stage_a(0)
                        for s in range(NS):
                            if s + 1 < NS:
                                stage_a(s + 1)
                            h, u = stage_b(s)
                            stage_rest(s, h, u)
                    else:
                        for s in range(NS):
                            stage_a(s)
                            h, u = stage_b(s)
                            stage_rest(s, h, u)

                for rep in range(repeat):
                    # ---------------- init ------------------------------
                    nc.vector.memset(accs[:], 0.0)
                    nc.sync.dma_start(eB[:, 0, :, 2 : W + 2], pred_r)
                    nc.sync.dma_start(eB[:, 1, :, 2 : W + 2], targ_r)
                    for s in range(NS):
                        cs = T * s + 2
                        # E/F strip partials land in cols 24+s / 28+s
                        nc.scalar.activation(
                            eA[:, 0, :, cs : cs + T], eB[:, 0, :, cs : cs + T],
                            AF.Sigmoid, accum_out=accs[:, 8 + 5 * NS + s : 9 + 5 * NS + s],
                        )
                        nc.scalar.activation(
                            eA[:, 1, :, cs : cs + T], eB[:, 1, :, cs : cs + T],
                            AF.Copy, accum_out=accs[:, 8 + 4 * NS + s : 9 + 4 * NS + s],
                        )
                    # pads: eA feeds the min pass (+S); eB feeds the max pass (-S)
                    nc.vector.memset(eA[:, :, :, 0:2], SENT)
                    nc.vector.memset(eA[:, :, :, W + 2 : W + 4], SENT)
                    nc.vector.memset(eB[:, :, :, 0:2], -SENT)
                    nc.vector.memset(eB[:, :, :, W + 2 : W + 4], -SENT)

                    # ---------------- skeleton rounds -------------------
                    # deferred w-multiplies: mt tiles from round i-1 are
                    # folded into w while round i's min pass runs, so the
                    # slow Pool-engine STT never blocks DVE directly.
                    upd_defer = os.environ.get("CLDICE_DEFER", "0") == "1"
                    pending = []

                    def flush_pending():
                        for mt_t, cs_t in pending:
                            nc.vector.tensor_tensor(
                                out=wbuf[:, :, :, cs_t : cs_t + T],
                                in0=wbuf[:, :, :, cs_t : cs_t + T],
                                in1=mt_t[:], op=AL.mult,
                            )
                        pending.clear()

                    cur, nxt = eA, eB
                    for i in range(rounds):
                        # erosion: nxt = minpool3(cur)
                        def min_dst(s, nxt=nxt):
                            return nxt[:, :, :, T * s + 2 : T * s + T + 2]

                        def min_post(s):
                            if pending:
                                mt_t, cs_t = pending.pop(0)
                                nc.vector.tensor_tensor(
                                    out=wbuf[:, :, :, cs_t : cs_t + T],
                                    in0=wbuf[:, :, :, cs_t : cs_t + T],
                                    in1=mt_t[:], op=AL.mult,
                                )

                        pool_pass(AL.min, cur, min_dst, SENT,
                                  post=min_post if upd_defer else None)

                        # opening: o = maxpool3(nxt); fold the w-update into
                        # the pass so each o strip is consumed immediately:
                        # w *= 1 + o - e   (e = cur, pre-erosion)
                        o_strips = [None] * NS

                        def max_dst(s, o_strips=o_strips):
                            o = scr2.tile([P, CH, R, T], dt, tag="o", name="o")
                            o_strips[s] = o
                            return o[:]

                        upd_gps = os.environ.get("CLDICE_UPD", "act") == "gps"

                        def upd(s, i=i, cur=cur, o_strips=o_strips):
                            cs = T * s
                            # w *= 1 + o - e   (st = o - e; mt = st + 1)
                            upd_eng = os.environ.get("CLDICE_UPD", "act")
                            wm_eng = os.environ.get("CLDICE_WMUL", "dve")
                            st = scr2.tile([P, CH, R, T], dt, tag="xs", name="st")
                            st_tt = (
                                nc.gpsimd.tensor_tensor
                                if upd_eng == "pool" else nc.vector.tensor_tensor
                            )
                            st_tt(
                                out=st[:], in0=o_strips[s][:],
                                in1=cur[:, :, :, cs + 2 : cs + T + 2],
                                op=AL.subtract,
                            )
                            if i == 0:
                                nc.scalar.activation(
                                    wbuf[:, :, :, cs : cs + T], st[:],
                                    AF.Copy, bias=1.0,
                                )
                                return
                            mt = mtp.tile([P, CH, R, T], dt, tag="mt", name="mt")
                            nc.scalar.activation(mt[:], st[:], AF.Copy, bias=1.0)
                            wm_tt = (
                                nc.gpsimd.tensor_tensor
                                if wm_eng == "gps" else nc.vector.tensor_tensor
                            )
                            wm_tt(
                                out=wbuf[:, :, :, cs : cs + T],
                                in0=wbuf[:, :, :, cs : cs + T],
                                in1=mt[:], op=AL.mult,
                            )

                        pool_pass(AL.max, nxt, max_dst, -SENT, post=upd)
                        if i < rounds - 1:
                            # pad flips: nxt (now holding e') feeds the next
                            # min pass (+S); cur becomes the next max-pass
                            # source (-S)
                            nc.gpsimd.memset(nxt[:, :, :, 0:2], SENT)
                            nc.gpsimd.memset(nxt[:, :, :, W + 2 : W + 4], SENT)
                            nc.gpsimd.memset(cur[:, :, :, 0:2], -SENT)
                            nc.gpsimd.memset(cur[:, :, :, W + 2 : W + 4], -SENT)
                        cur, nxt = nxt, cur
                    flush_pending()

                    # ---------------- final sums ------------------------
                    # accs strip-partial columns (combined on the host):
                    #  A=sum(w_p*t16): 8+s   B=sum(w_p): 12+s
                    #  C=sum(w_t*p16): 16+s  D=sum(w_t): 20+s
                    #  E=sum(t16): 24+s      F=sum(p16): 28+s  (from init)
                    # re-stream the images (e tiles are dead now):
                    # eB ch0 <- pred, ch1 <- target (= t16); p16 recomputed
                    # strip-wise into eA ch0.
                    nc.sync.dma_start(eB[:, 0, :, 2 : W + 2], pred_r)
                    nc.sync.dma_start(eB[:, 1, :, 2 : W + 2], targ_r)
                    for s in range(NS):
                        cs = T * s
                        nc.vector.tensor_reduce(
                            out=accs[:, 8 + NS + s : 9 + NS + s],
                            in_=wbuf[:, 0, :, cs : cs + T],
                            axis=mybir.AxisListType.XY, op=AL.add,
                        )
                        nc.vector.tensor_reduce(
                            out=accs[:, 8 + 3 * NS + s : 9 + 3 * NS + s],
                            in_=wbuf[:, 1, :, cs : cs + T],
                            axis=mybir.AxisListType.XY, op=AL.add,
                        )
                        nc.scalar.activation(
                            eA[:, 0, :, cs + 2 : cs + T + 2],
                            eB[:, 0, :, cs + 2 : cs + T + 2], AF.Sigmoid,
                        )
                        prod = scr2.tile([P, CH, R, T], dt, tag="m1", name="prod")
                        nc.vector.tensor_tensor(
                            out=prod[:, 0, :, :],
                            in0=wbuf[:, 0, :, cs : cs + T],
                            in1=eB[:, 1, :, cs + 2 : cs + T + 2],
                            op=AL.mult,
                        )
                        nc.vector.tensor_tensor(
                            out=prod[:, 1, :, :],
                            in0=wbuf[:, 1, :, cs : cs + T],
                            in1=eA[:, 0, :, cs + 2 : cs + T + 2],
                            op=AL.mult,
                        )
                        junk = scr2.tile([P, CH, R, T], dt, tag="xs", name="junk")
                        nc.scalar.activation(
                            junk[:, 0, :, :], prod[:, 0, :, :], AF.Copy,
                            accum_out=accs[:, 8 + s : 9 + s],
                        )
                        nc.scalar.activation(
                            junk[:, 1, :, :], prod[:, 1, :, :], AF.Copy,
                            accum_out=accs[:, 8 + 2 * NS + s : 9 + 2 * NS + s],
                        )

                    with tc.tile_pool(name="psf", bufs=1, space="PSUM") as psf:
                        ps = psf.tile([1, ACCW], fp32, name="psf")
                        nc.tensor.matmul(ps[:], ones[:], accs[:], start=True, stop=True)
                        nc.vector.tensor_copy(redout[0:1, :], ps[:])
                    nc.sync.dma_start(out_d[:], redout[0:1, :])

    return nc


def _get_built(H=1024, W=1024, rounds=None):
    if rounds is None:
        rounds = int(os.environ.get("CLDICE_ROUNDS", str(NUM_ITER + 1)))
    key = (H, W, rounds)
    if key not in _BUILT:
        _BUILT[key] = build_nc(H, W, rounds=rounds)
    return _BUILT[key]


def kernel(pred: np.ndarray, target: np.ndarray) -> np.ndarray:
    """Full-input entry point: pred/target [8,1,1024,1024] f32 -> scalar."""
    from concourse.bass_utils import run_bass_kernel_spmd

    n_cores = pred.shape[0]
    nc = _get_built(pred.shape[2], pred.shape[3])
    in_maps = [
        {
            "pred": np.ascontiguousarray(pred[c, 0], dtype=np.float16),
            "target": np.ascontiguousarray(target[c, 0], dtype=np.float16),
        }
        for c in range(n_cores)
    ]
    res = run_bass_kernel_spmd(nc, in_maps, list(range(n_cores)))
    outs = np.stack([res.results[c]["out"][0] for c in range(n_cores)])  # [8,32]
    return _combine(outs, pred.shape[2] * pred.shape[3])


def _combine(outs: np.ndarray, n_per_core: int) -> np.ndarray:
    o = outs.astype(np.float64)
    ns = (o.shape[1] - 8) // 6
    A, B, C, D, E, F = (
        o[:, 8 + k * ns : 8 + (k + 1) * ns].sum(axis=1) for k in range(6)
    )
    S1 = np.sum(E - A)  # sum(skel_pred * target)
    S2 = np.sum(n_per_core - B)  # sum(skel_pred)
    S3 = np.sum(F - C)  # sum(skel_target * pred_prob)
    S4 = np.sum(n_per_core - D)  # sum(skel_target)
    tprec = (S1 + SMOOTH) / (S2 + SMOOTH)
    tsens = (S3 + SMOOTH) / (S4 + SMOOTH)
    cl_dice = 2.0 * tprec * tsens / (tprec + tsens + EPS)
    return np.float32(1.0 - cl_dice)


# revision 5
# speedup vs baseline: 68.4101x; 1.1231x over previous
"""CenterlineDiceLoss (soft-skeleton clDice) Trainium2 Bass kernel, v2.

Data-parallel over the batch (8 images -> 8 NeuronCores).  Each core runs
both soft-skeleton chains (sigmoid(pred), target) fully SBUF-resident in
fp16 with ZERO DMAs inside the iteration loop:

 - The two chains are fused along the free dimension ([P, CH=2, R, W]
   tiles) so every engine instruction processes both chains at once.
 - 3x3 min/max pools are separable pairwise ops on DVE.  The one-element
   shifted operands read misaligned fp16 directly: measured on HW this
   costs only ~40% over the aligned 2x mode and beats staging shifted
   copies through ScalarE/DMA (those serialize the dependency chain).
   Edge/interior op splitting keeps the PE/ACT halo path off the DVE
   critical path; DVE occupancy is ~98% in the cost-model timeline.
 - Cross-partition row halos are produced by PE shift-matmuls
   (permutation matrices built once with affine_select) into PSUM and
   evacuated by ACT - no SBUF->SBUF partition-shifted DMAs at all.
 - The skeleton recurrence is tracked in complement space w = 1 - skel:
   w *= (1 + o - e), computed as s = o - e (DVE), m = s + 1 (ACT bias
   copy), w *= m (DVE).
 - Final global sums reduce on-chip (ACT accum + DVE reduce + PE
   ones-matmul) to one [1, 32] fp32 vector per core; the host combines
   the 8 vectors into the scalar loss.  The input images (needed for the
   cross products) are re-streamed from DRAM at that point rather than
   held in SBUF through the rounds.
"""

import os
import numpy as np

NUM_ITER = 10
SMOOTH = 1.0
EPS = 1e-7
SENT = 30000.0  # pad sentinel (exactly representable in fp16)

_BUILT = {}


def _install_walrus_wait_patch():
    """This container's walrus rejects >1 sync-wait per instruction; split
    extra waits onto NoOp/Drain instructions on the same engine."""
    import concourse.tile as tile_mod
    import mybir

    if getattr(tile_mod.TileContext, "_cldice_patched", False):
        return

    _orig_add_instruction = tile_mod.TileContext._add_instruction
    _ctr = [0]

    def _patched_add_instruction(self, inst):
        si = getattr(inst, "sync_info", None)
        if (
            si is not None
            and si.on_wait is not None
            and len(si.on_wait) > 1
            and inst.engine != mybir.EngineType.Unassigned
        ):
            waits = list(si.on_wait)
            ups = list(si.on_update) if si.on_update else []
            for w in waits[:-1]:
                _ctr[0] += 1
                nop = mybir.InstNoOp(
                    name=f"{inst.name}_sw{_ctr[0]}",
                    sync_info=mybir.SyncInfo(on_wait=[w], on_update=[]),
                    bass_nofuse=True,
                    engine=inst.engine,
                )
                _orig_add_instruction(self, nop)
            inst.sync_info = mybir.SyncInfo(on_wait=waits[-1:], on_update=ups)
        return _orig_add_instruction(self, inst)

    def _patched_drain_and_barrier(self, tick_clock, wait_clock):
        nc = self.nc
        drain_inst = nc.sync.drain()
        wait_clock.add_sem_waits(
            drain_inst.ins, tile_mod.ScopedClock({None: tick_clock.global_clock})
        )
        si = drain_inst.ins.sync_info
        if si is not None and si.on_wait is not None and len(si.on_wait) > 1:
            waits = list(si.on_wait)
            ups = list(si.on_update) if si.on_update else []
            drain_inst.ins.sync_info = mybir.SyncInfo(on_wait=waits[:1], on_update=[])
            for w in waits[1:]:
                extra = nc.sync.drain()
                extra.ins.sync_info = mybir.SyncInfo(on_wait=[w], on_update=[])
            if ups:
                extra2 = nc.sync.drain()
                extra2.ins.sync_info = mybir.SyncInfo(on_wait=[], on_update=ups)
        nc.all_engine_barrier()
        assert self.sems is not None
        popped = nc._tile_sem_poison_stack.pop()
        assert popped is self._sem_poison
        nc.clear_and_free_semaphores(list(self.sems.allocated().values()))
        nc.all_engine_barrier()

    tile_mod.TileContext._add_instruction = _patched_add_instruction
    tile_mod.TileContext._drain_and_barrier = _patched_drain_and_barrier
    tile_mod.TileContext._cldice_patched = True


def build_nc(H=1024, W=1024, rounds=NUM_ITER + 1, repeat=1, T=None):
    """Build the single-core Bass program (run SPMD across 8 cores)."""
    import concourse.bass as bass
    import concourse.tile as tile
    import mybir

    _install_walrus_wait_patch()

    P = 128
    R = H // P          # image rows per partition (8)
    CH = 2              # fused chains: 0 = sigmoid(pred), 1 = target
    WB = W + 4          # padded row: cols 0..1 pad, 2..W+1 image, W+2..W+3 pad
    if T is None:
        T = int(os.environ.get("CLDICE_T", "256"))
    NS = W // T
    fp32 = mybir.dt.float32
    dt = mybir.dt.float16
    AL = mybir.AluOpType
    AF = mybir.ActivationFunctionType

    ACCW = 8 + 6 * NS   # strip-partial columns: A,B,C,D,E,F groups of NS
    nc = bass.Bass("TRN2", target_bir_lowering=False, debug=False)
    pred_d = nc.dram_tensor("pred", [H, W], dt, kind="ExternalInput").ap()
    targ_d = nc.dram_tensor("target", [H, W], dt, kind="ExternalInput").ap()
    out_d = nc.dram_tensor("out", [1, ACCW], fp32, kind="ExternalOutput").ap()
    pred_r = pred_d.rearrange("(p j) c -> p j c", p=P)
    targ_r = targ_d.rearrange("(p j) c -> p j c", p=P)

    with tile.TileContext(nc) as tc:
        with tc.tile_pool(name="persist", bufs=1) as pp:
            eA = pp.tile([P, CH, R, WB], dt, tag="eA", name="eA")
            eB = pp.tile([P, CH, R, WB], dt, tag="eB", name="eB")
            wbuf = pp.tile([P, CH, R, W], dt, tag="w", name="w")
            accs = pp.tile([P, ACCW], fp32, tag="accs")
            redout = pp.tile([P, ACCW], fp32, tag="redout")
            ones = pp.tile([P, 1], fp32, tag="ones", name="ones")
            ones16 = pp.tile([P, P], dt, tag="ones16", name="ones16")
            # shift matrices (lhsT for matmul: out = lhsT.T @ rhs):
            # sd: out[m] = rhs[m-1]  (halo_top[p] <- row from partition p-1)
            # su: out[m] = rhs[m+1]  (halo_bot[p] <- row from partition p+1)
            sd = pp.tile([P, P], dt, tag="sd", name="sd")
            su = pp.tile([P, P], dt, tag="su", name="su")
            # edge-sentinel matmul operands: eT has a single 1 at (k=0, m=0),
            # eB at (k=0, m=127); eT.T @ sentX adds sentinel into out row 0
            # (partition 0), eB.T @ sentX into partition 127.
            eT = pp.tile([P, P], dt, tag="eT", name="eT")
            eB_m = pp.tile([P, P], dt, tag="eBm", name="eBm")
            sentP = pp.tile([P, T_MAX := 512], dt, tag="sentP", name="sentP")
            sentN = pp.tile([P, T_MAX], dt, tag="sentN", name="sentN")

            nc.vector.memset(ones[:], 1.0)
            nc.vector.memset(ones16[:], 1.0)
            nc.vector.memset(sentP[:], SENT)
            nc.vector.memset(sentN[:], -SENT)
            # lhsT[k, m] = 1 iff m == k+1   (iota = -1 - k + m == 0)
            nc.gpsimd.affine_select(
                sd[:], ones16[:], pattern=[[1, P]], compare_op=AL.is_equal,
                fill=0.0, base=-1, channel_multiplier=-1,
            )
            # lhsT[k, m] = 1 iff m == k-1   (iota = 1 - k + m == 0)
            nc.gpsimd.affine_select(
                su[:], ones16[:], pattern=[[1, P]], compare_op=AL.is_equal,
                fill=0.0, base=1, channel_multiplier=-1,
            )
            # 1 iff k + m == 0  (only k=0, m=0)
            nc.gpsimd.affine_select(
                eT[:], ones16[:], pattern=[[1, P]], compare_op=AL.is_equal,
                fill=0.0, base=0, channel_multiplier=1,
            )
            # 1 iff 127 + k - m == 0  (only k=0, m=127)
            nc.gpsimd.affine_select(
                eB_m[:], ones16[:], pattern=[[-1, P]], compare_op=AL.is_equal,
                fill=0.0, base=P - 1, channel_multiplier=1,
            )

            scr1_bufs = int(os.environ.get("CLDICE_SCR1_BUFS", "2"))
            B = lambda k, d: int(os.environ.get(k, d))
            with tc.tile_pool(name="scr1", bufs=scr1_bufs) as scr1, \
                 tc.tile_pool(name="m1p", bufs=B("CLDICE_B_M1", "2")) as m1p, \
                 tc.tile_pool(name="stp", bufs=B("CLDICE_B_ST", "2")) as stp, \
                 tc.tile_pool(name="op_", bufs=B("CLDICE_B_O", "2")) as op_, \
                 tc.tile_pool(name="cpp", bufs=B("CLDICE_B_CP", "3")) as cpp, \
                 tc.tile_pool(name="mtp", bufs=int(os.environ.get("CLDICE_MTP", "2"))) as mtp, \
                 tc.tile_pool(name="psum", bufs=2 if CH * T <= 512 else 1,
                              space="PSUM") as psp:

                xs_dma = os.environ.get("CLDICE_XS", "none") == "dma"
                h_odd = os.environ.get("CLDICE_H", "m1s") == "odd"
                pipe = os.environ.get("CLDICE_PIPE", "0") == "1"

                def pool_pass(op, src, dst_of_strip, sent, post=None):
                    """3x3 pool of padded `src` [P,CH,R,WB] with `op`;
                    dst_of_strip(s) -> [P,CH,R,T] output AP for strip s.
                    `sent`: sentinel for the out-of-image row halos.
                    `post(s)`: emitted after strip s's output is ready.
                    Emission is optionally software-pipelined (stage A =
                    xs+m1, stage B = rest) so DVE has ready work while the
                    shifted copies land."""
                    stash = {}

                    def stage_a(s):
                        cs = T * s
                        m1 = m1p.tile([P, CH, R, T + 2], dt, tag="m1", name="m1")
                        # horizontal: out[c] = op(x[c-1], x[c], x[c+1])
                        xs_mode = os.environ.get("CLDICE_XS", "none")
                        if xs_mode == "none":
                            # direct misaligned read (~0.85 cyc/elem on HW)
                            xs = None
                            nc.vector.tensor_tensor(
                                out=m1[:], in0=src[:, :, :, cs : cs + T + 2],
                                in1=src[:, :, :, cs + 1 : cs + T + 3], op=op,
                            )
                        else:
                            xs = cpp.tile([P, CH, R, T + 2], dt, tag="xs",
                                           name="xs")
                            if xs_mode == "dma":
                                nc.sync.dma_start(
                                    xs[:], src[:, :, :, cs + 1 : cs + T + 3]
                                )
                            else:
                                nc.scalar.activation(
                                    xs[:], src[:, :, :, cs + 1 : cs + T + 3],
                                    AF.Copy,
                                )
                            nc.vector.tensor_tensor(
                                out=m1[:], in0=src[:, :, :, cs : cs + T + 2],
                                in1=xs[:], op=op,
                            )
                        stash[s] = (xs, m1)

                    def stage_b(s):
                        xs, m1 = stash.pop(s)
                        h = scr1.tile([P, CH, R + 2, T], dt, tag="h", name="h")
                        u = scr1.tile([P, CH, R + 1, T], dt, tag="u", name="u")
                        m1s_mode = os.environ.get("CLDICE_M1S", "none")
                        usplit = int(os.environ.get("CLDICE_USPLIT", "2"))
                        if m1s_mode == "none" and usplit >= 2:
                            # edge rows {1, R} first so the PE halo matmuls
                            # start while the interior rows compute
                            nc.vector.tensor_tensor(
                                out=h[:, :, 1 : R + 1 : R - 1, :],
                                in0=m1[:, :, 0 : R : R - 1, 2 : T + 2],
                                in1=m1[:, :, 0 : R : R - 1, 1 : T + 1], op=op,
                            )
                            nc.vector.tensor_tensor(
                                out=h[:, :, 2 : R, :],
                                in0=m1[:, :, 1 : R - 1, 2 : T + 2],
                                in1=m1[:, :, 1 : R - 1, 1 : T + 1], op=op,
                            )
                        elif m1s_mode == "none":
                            # direct misaligned read of m1
                            nc.vector.tensor_tensor(
                                out=h[:, :, 1 : R + 1, :],
                                in0=m1[:, :, :, 2 : T + 2],
                                in1=m1[:, :, :, 1 : T + 1], op=op,
                            )
                        elif h_odd:
                            nc.vector.tensor_tensor(
                                out=h[:, :, 1 : R + 1, :],
                                in0=m1[:, :, :, 1 : T + 1],
                                in1=xs[:, :, :, 2 : T + 2], op=op,
                            )
                        else:
                            m1s = cpp.tile([P, CH, R, T], dt, tag="m1s",
                                            name="m1s")
                            if os.environ.get("CLDICE_M1S", "none") == "dma":
                                nc.sync.dma_start(
                                    m1s[:, 0, :, :], m1[:, 0, :, 1 : T + 1]
                                )
                                nc.sync.dma_start(
                                    m1s[:, 1, :, :], m1[:, 1, :, 1 : T + 1]
                                )
                            else:
                                nc.scalar.activation(
                                    m1s[:], m1[:, :, :, 1 : T + 1], AF.Copy
                                )
                            nc.vector.tensor_tensor(
                                out=h[:, :, 1 : R + 1, :],
                                in0=m1[:, :, :, 2 : T + 2], in1=m1s[:], op=op,
                            )
                        return h, u

                    def stage_rest(s, h, u):
                        cs = T * s
                        # cross-partition halo rows via PE shift-matmuls,
                        # both channels per matmul (out free = CH*T <= 512).
                        # hp[:, 0] = top halo, hp[:, 1] = bottom; a second
                        # accumulating matmul adds the sentinel into the
                        # image-edge partitions (0 / 127); ACT evacuates
                        # into h rows {0, R+1}.
                        sent_t = sentP if sent > 0 else sentN
                        hp = psp.tile([P, 2, CH, T], fp32, tag="hp", name="hp")
                        if CH * T <= 512:
                            mm_groups = [(hp[:, 0, :, :], sd, h[:, :, R : R + 1, :]),
                                         (hp[:, 1, :, :], su, h[:, :, 1:2, :])]
                            sent_mats = [eT, eB_m]
                            for (dst, mat, src_rows), emat in zip(
                                mm_groups, sent_mats
                            ):
                                nc.tensor.matmul(
                                    dst, mat[:], src_rows, start=True, stop=False
                                )
                                nc.tensor.matmul(
                                    dst, emat[:], sent_t[:, 0 : CH * T],
                                    start=False, stop=True,
                                )
                        else:
                            for ch in range(CH):
                                for d, mat, emat, row in (
                                    (0, sd, eT, R), (1, su, eB_m, 1),
                                ):
                                    nc.tensor.matmul(
                                        hp[:, d, ch, :], mat[:],
                                        h[:, ch, row : row + 1, :],
                                        start=True, stop=False,
                                    )
                                    nc.tensor.matmul(
                                        hp[:, d, ch, :], emat[:],
                                        sent_t[:, 0:T],
                                        start=False, stop=True,
                                    )
                        nc.scalar.activation(h[:, :, 0:1, :], hp[:, 0, :, :], AF.Copy)
                        nc.scalar.activation(
                            h[:, :, R + 1 : R + 2, :], hp[:, 1, :, :], AF.Copy
                        )
                        # vertical: out[r] = op(h[r-1], h[r], h[r+1])
                        usplit2 = int(os.environ.get("CLDICE_USPLIT", "2"))
                        if usplit2 >= 1:
                            # interior rows don't need the halos -> no PE/ACT
                            # wait on the critical path
                            nc.vector.tensor_tensor(
                                out=u[:, :, 1:R, :], in0=h[:, :, 1:R, :],
                                in1=h[:, :, 2 : R + 1, :], op=op,
                            )
                            nc.vector.tensor_tensor(
                                out=u[:, :, 0 : R + 1 : R, :],
                                in0=h[:, :, 0 : R + 1 : R, :],
                                in1=h[:, :, 1 : R + 2 : R, :], op=op,
                            )
                        else:
                            nc.vector.tensor_tensor(
                                out=u[:], in0=h[:, :, 0 : R + 1, :],
                                in1=h[:, :, 1 : R + 2, :], op=op,
                            )
                        nc.vector.tensor_tensor(
                            out=dst_of_strip(s), in0=u[:, :, 0:R, :],
                            in1=u[:, :, 1 : R + 1, :], op=op,
                        )
                        if post is not None:
                            post(s)

                    if pipe:
                        stage_a(0)
                        for s in range(NS):
                            if s + 1 < NS:
                                stage_a(s + 1)
                            h, u = stage_b(s)
                            stage_rest(s, h, u)
                    else:
                        for s in range(NS):
                            stage_a(s)
                            h, u = stage_b(s)
                            stage_rest(s, h, u)

                for rep in range(repeat):
                    # ---------------- init ------------------------------
                    nc.vector.memset(accs[:], 0.0)
                    nc.sync.dma_start(eB[:, 0, :, 2 : W + 2], pred_r)
                    nc.sync.dma_start(eB[:, 1, :, 2 : W + 2], targ_r)
                    for s in range(NS):
                        cs = T * s + 2
                        # E/F strip partials land in cols 24+s / 28+s
                        nc.scalar.activation(
                            eA[:, 0, :, cs : cs + T], eB[:, 0, :, cs : cs + T],
                            AF.Sigmoid, accum_out=accs[:, 8 + 5 * NS + s : 9 + 5 * NS + s],
                        )
                        nc.scalar.activation(
                            eA[:, 1, :, cs : cs + T], eB[:, 1, :, cs : cs + T],
                            AF.Copy, accum_out=accs[:, 8 + 4 * NS + s : 9 + 4 * NS + s],
                        )
                    # pads: eA feeds the min pass (+S); eB feeds the max pass (-S)
                    nc.vector.memset(eA[:, :, :, 0:2], SENT)
                    nc.vector.memset(eA[:, :, :, W + 2 : W + 4], SENT)
                    nc.vector.memset(eB[:, :, :, 0:2], -SENT)
                    nc.vector.memset(eB[:, :, :, W + 2 : W + 4], -SENT)

                    # ---------------- skeleton rounds -------------------
                    # deferred w-multiplies: mt tiles from round i-1 are
                    # folded into w while round i's min pass runs, so the
                    # slow Pool-engine STT never blocks DVE directly.
                    upd_defer = os.environ.get("CLDICE_DEFER", "0") == "1"
                    pending = []

                    def flush_pending():
                        for mt_t, cs_t in pending:
                            nc.vector.tensor_tensor(
                                out=wbuf[:, :, :, cs_t : cs_t + T],
                                in0=wbuf[:, :, :, cs_t : cs_t + T],
                                in1=mt_t[:], op=AL.mult,
                            )
                        pending.clear()

                    cur, nxt = eA, eB
                    for i in range(rounds):
                        # erosion: nxt = minpool3(cur)
                        def min_dst(s, nxt=nxt):
                            return nxt[:, :, :, T * s + 2 : T * s + T + 2]

                        def min_post(s):
                            if pending:
                                mt_t, cs_t = pending.pop(0)
                                nc.vector.tensor_tensor(
                                    out=wbuf[:, :, :, cs_t : cs_t + T],
                                    in0=wbuf[:, :, :, cs_t : cs_t + T],
                                    in1=mt_t[:], op=AL.mult,
                                )

                        pool_pass(AL.min, cur, min_dst, SENT,
                                  post=min_post if upd_defer else None)

                        # opening: o = maxpool3(nxt); fold the w-update into
                        # the pass so each o strip is consumed immediately:
                        # w *= 1 + o - e   (e = cur, pre-erosion)
                        o_strips = [None] * NS

                        def max_dst(s, o_strips=o_strips):
                            o = op_.tile([P, CH, R, T], dt, tag="o", name="o")
                            o_strips[s] = o
                            return o[:]

                        upd_gps = os.environ.get("CLDICE_UPD", "act") == "gps"

                        def upd(s, i=i, cur=cur, o_strips=o_strips):
                            cs = T * s
                            # w *= 1 + o - e   (st = o - e; mt = st + 1)
                            upd_eng = os.environ.get("CLDICE_UPD", "act")
                            wm_eng = os.environ.get("CLDICE_WMUL", "dve")
                            st = stp.tile([P, CH, R, T], dt, tag="st", name="st")
                            st_tt = (
                                nc.gpsimd.tensor_tensor
                                if upd_eng == "pool" else nc.vector.tensor_tensor
                            )
                            st_tt(
                                out=st[:], in0=o_strips[s][:],
                                in1=cur[:, :, :, cs + 2 : cs + T + 2],
                                op=AL.subtract,
                            )
                            if i == 0:
                                nc.scalar.activation(
                                    wbuf[:, :, :, cs : cs + T], st[:],
                                    AF.Copy, bias=1.0,
                                )
                                return
                            if os.environ.get("CLDICE_MT", "mtp") == "inplace":
                                nc.scalar.activation(st[:], st[:], AF.Copy,
                                                     bias=1.0)
                                mt = st
                            else:
                                mt = mtp.tile([P, CH, R, T], dt, tag="mt",
                                              name="mt")
                                nc.scalar.activation(mt[:], st[:], AF.Copy,
                                                     bias=1.0)
                            wm_tt = (
                                nc.gpsimd.tensor_tensor
                                if wm_eng == "gps" else nc.vector.tensor_tensor
                            )
                            wm_tt(
                                out=wbuf[:, :, :, cs : cs + T],
                                in0=wbuf[:, :, :, cs : cs + T],
                                in1=mt[:], op=AL.mult,
                            )

                        pool_pass(AL.max, nxt, max_dst, -SENT, post=upd)
                        if i < rounds - 1:
                            # pad flips: nxt (now holding e') feeds the next
                            # min pass (+S); cur becomes the next max-pass
                            # source (-S)
                            nc.gpsimd.memset(nxt[:, :, :, 0:2], SENT)
                            nc.gpsimd.memset(nxt[:, :, :, W + 2 : W + 4], SENT)
                            nc.gpsimd.memset(cur[:, :, :, 0:2], -SENT)
                            nc.gpsimd.memset(cur[:, :, :, W + 2 : W + 4], -SENT)
                        cur, nxt = nxt, cur
                    flush_pending()

                    # ---------------- final sums ------------------------
                    # accs strip-partial columns (combined on the host):
                    #  A=sum(w_p*t16): 8+s   B=sum(w_p): 12+s
                    #  C=sum(w_t*p16): 16+s  D=sum(w_t): 20+s
                    #  E=sum(t16): 24+s      F=sum(p16): 28+s  (from init)
                    # re-stream the images (e tiles are dead now):
                    # eB ch0 <- pred, ch1 <- target (= t16); p16 recomputed
                    # strip-wise into eA ch0.
                    nc.sync.dma_start(eB[:, 0, :, 2 : W + 2], pred_r)
                    nc.sync.dma_start(eB[:, 1, :, 2 : W + 2], targ_r)
                    for s in range(NS):
                        cs = T * s
                        # B/D strip sums via ACT accumulate-copies (ACT is
                        # idle here; keeps DVE free for the prod TTs)
                        wsum = stp.tile([P, CH, R, T], dt, tag="st", name="wsum")
                        nc.scalar.activation(
                            wsum[:, 0, :, :], wbuf[:, 0, :, cs : cs + T],
                            AF.Copy, accum_out=accs[:, 8 + NS + s : 9 + NS + s],
                        )
                        nc.scalar.activation(
                            wsum[:, 1, :, :], wbuf[:, 1, :, cs : cs + T],
                            AF.Copy,
                            accum_out=accs[:, 8 + 3 * NS + s : 9 + 3 * NS + s],
                        )
                        nc.scalar.activation(
                            eA[:, 0, :, cs + 2 : cs + T + 2],
                            eB[:, 0, :, cs + 2 : cs + T + 2], AF.Sigmoid,
                        )
                        prod = m1p.tile([P, CH, R, T], dt, tag="m1", name="prod")
                        nc.vector.tensor_tensor(
                            out=prod[:, 0, :, :],
                            in0=wbuf[:, 0, :, cs : cs + T],
                            in1=eB[:, 1, :, cs + 2 : cs + T + 2],
                            op=AL.mult,
                        )
                        nc.vector.tensor_tensor(
                            out=prod[:, 1, :, :],
                            in0=wbuf[:, 1, :, cs : cs + T],
                            in1=eA[:, 0, :, cs + 2 : cs + T + 2],
                            op=AL.mult,
                        )
                        junk = stp.tile([P, CH, R, T], dt, tag="st", name="junk")
                        nc.scalar.activation(
                            junk[:, 0, :, :], prod[:, 0, :, :], AF.Copy,
                            accum_out=accs[:, 8 + s : 9 + s],
                        )
                        nc.scalar.activation(
                            junk[:, 1, :, :], prod[:, 1, :, :], AF.Copy,
                            accum_out=accs[:, 8 + 2 * NS + s : 9 + 2 * NS + s],
                        )

                    with tc.tile_pool(name="psf", bufs=1, space="PSUM") as psf:
                        ps = psf.tile([1, ACCW], fp32, name="psf")
                        nc.tensor.matmul(ps[:], ones[:], accs[:], start=True, stop=True)
                        nc.vector.tensor_copy(redout[0:1, :], ps[:])
                    nc.sync.dma_start(out_d[:], redout[0:1, :])

    return nc


def _get_built(H=1024, W=1024, rounds=None):
    if rounds is None:
        rounds = int(os.environ.get("CLDICE_ROUNDS", str(NUM_ITER + 1)))
    key = (H, W, rounds)
    if key not in _BUILT:
        _BUILT[key] = build_nc(H, W, rounds=rounds)
    return _BUILT[key]


def kernel(pred: np.ndarray, target: np.ndarray) -> np.ndarray:
    """Full-input entry point: pred/target [8,1,1024,1024] f32 -> scalar."""
    from concourse.bass_utils import run_bass_kernel_spmd

    n_cores = pred.shape[0]
    nc = _get_built(pred.shape[2], pred.shape[3])
    in_maps = [
        {
            "pred": np.ascontiguousarray(pred[c, 0], dtype=np.float16),
            "target": np.ascontiguousarray(target[c, 0], dtype=np.float16),
        }
        for c in range(n_cores)
    ]
    res = run_bass_kernel_spmd(nc, in_maps, list(range(n_cores)))
    outs = np.stack([res.results[c]["out"][0] for c in range(n_cores)])  # [8,32]
    return _combine(outs, pred.shape[2] * pred.shape[3])


def _combine(outs: np.ndarray, n_per_core: int) -> np.ndarray:
    o = outs.astype(np.float64)
    ns = (o.shape[1] - 8) // 6
    A, B, C, D, E, F = (
        o[:, 8 + k * ns : 8 + (k + 1) * ns].sum(axis=1) for k in range(6)
    )
    S1 = np.sum(E - A)  # sum(skel_pred * target)
    S2 = np.sum(n_per_core - B)  # sum(skel_pred)
    S3 = np.sum(F - C)  # sum(skel_target * pred_prob)
    S4 = np.sum(n_per_core - D)  # sum(skel_target)
    tprec = (S1 + SMOOTH) / (S2 + SMOOTH)
    tsens = (S3 + SMOOTH) / (S4 + SMOOTH)
    cl_dice = 2.0 * tprec * tsens / (tprec + tsens + EPS)
    return np.float32(1.0 - cl_dice)
